# revision 32
# baseline (speedup 1.0000x reference)
"""Trainium2 Bass kernel for a pre-LN transformer block.

Shapes (hardcoded): x [4, 1024, 1024], D=1024, H=16 heads, DH=64, F=4096.

Sharding over 8 cores, no collectives: core c handles batch b=c//2 and
query-half c%2 (512 queries). Each core recomputes LN1 + K/V for all 1024
tokens of its batch element (duplicated within the pair - cheaper than an
all-reduce), and computes Q/attention/proj/MLP only for its own 512 rows.
Host reorders rows so each core's own queries are rows 0:512 (attention is
permutation-invariant over keys), making the program uniform across cores.

Numerics: bf16 matmul operands, fp32 PSUM accumulation, fp32 LayerNorm /
softmax / residuals. LN gain/bias are folded into the following weight
matrix on the host (algebraically exact). Softmax skips max-subtraction
(|scores| <= ~6 in fp32) and normalizes after attn@V using denominators
accumulated for free via a ones-column appended to V.
"""

import sys

try:
    import concourse.bass as bass
except ImportError:  # pragma: no cover
    sys.path.insert(0, "/opt/trn_rl_repo")
    import concourse.bass as bass

import numpy as np
import ml_dtypes

import concourse.mybir as mybir
import concourse.tile as tile
from concourse import bacc
from concourse.bass_utils import run_bass_kernel_spmd
from concourse.masks import make_identity

BF16 = mybir.dt.bfloat16
F32 = mybir.dt.float32
AF = mybir.ActivationFunctionType
OP = mybir.AluOpType

P = 128          # partitions
D = 1024         # model dim
H = 16           # heads
DH = 64          # head dim
F = 4096         # mlp hidden
N = 1024         # tokens (keys) per batch element
NQ = 512         # queries owned per core
NT = N // P      # 8 token tiles
NQT = NQ // P    # 4 own token tiles
KD = D // P      # 8 contraction tiles over D
FT = F // P      # 32 tiles over F
EPS = 1e-6
SCALE = DH ** -0.5

TRACE = False        # set True (e.g. from test.py) to capture an NTFF trace
LAST_RESULTS = None  # BassKernelResults of the most recent run


def _layer_norm_tiles(nc, pools, x_src_tiles, out_cb):
    """Emit LN over a list of [P, D] fp32 SBUF tiles.

    out_cb(i, h_bf16_tile) receives the normalized bf16 tile.
    """
    small = pools["small"]
    tmp = pools["lntmp"]
    eps_t = pools["eps"]
    for i, x_t in enumerate(x_src_tiles):
        xr = x_t.rearrange("p (s f) -> p s f", f=512)
        stats = small.tile([P, 2, 6], F32, tag="lnstats", name=f"lnstats_{i}")
        for s in range(2):
            nc.vector.bn_stats(out=stats[:, s, :], in_=xr[:, s, :])
        mv = small.tile([P, 2], F32, tag="lnmv", name=f"lnmv_{i}")
        nc.vector.bn_aggr(out=mv, in_=stats)
        std = small.tile([P, 1], F32, tag="lnstd", name=f"lnstd_{i}")
        nc.scalar.activation(out=std, in_=mv[:, 1:2], func=AF.Sqrt,
                             bias=eps_t, scale=1.0)
        rstd = small.tile([P, 1], F32, tag="lnrstd", name=f"lnrstd_{i}")
        nc.vector.reciprocal(rstd, std)
        h_t = tmp.tile([P, D], BF16, tag="lnh", name=f"lnh_{i}")
        nc.vector.tensor_scalar(out=h_t, in0=x_t, scalar1=mv[:, 0:1],
                                scalar2=rstd, op0=OP.subtract, op1=OP.mult)
        out_cb(i, h_t)


def _transpose_to(nc, tp_pool, ident, src_bf16, dst_cb, n_in_tiles=KD, idx0=0):
    """PE-transpose [P, n*128] bf16 tile into column blocks via dst_cb(j, ps)."""
    for j in range(n_in_tiles):
        ps = tp_pool.tile([P, P], BF16, tag="tp", name=f"tp_{idx0}_{j}")
        nc.tensor.transpose(ps, src_bf16[:, j * P:(j + 1) * P], ident)
        dst_cb(j, ps)


def build_program(with_biases=True):
    nc = bacc.Bacc("TRN2", debug=False, enable_asserts=False, num_devices=8)

    x_in = nc.dram_tensor("x_in", [N, D], F32, kind="ExternalInput").ap()
    wqkv = nc.dram_tensor("w_qkv", [D, 3 * D], BF16, kind="ExternalInput").ap()
    qkvb_col = nc.dram_tensor("qkv_b_col", [P, 3 * KD], F32, kind="ExternalInput").ap()
    qkvb_row = nc.dram_tensor("qkv_b_row", [1, 3 * D], BF16, kind="ExternalInput").ap()
    wproj = nc.dram_tensor("w_proj", [D, D], BF16, kind="ExternalInput").ap()
    bproj_row = nc.dram_tensor("b_proj_row", [1, D], BF16, kind="ExternalInput").ap()
    wfc1 = nc.dram_tensor("w_fc1", [D, F], BF16, kind="ExternalInput").ap()
    fc1b_col = nc.dram_tensor("fc1_b_col", [P, FT], F32, kind="ExternalInput").ap()
    wfc2 = nc.dram_tensor("w_fc2", [F, D], BF16, kind="ExternalInput").ap()
    bfc2_row = nc.dram_tensor("b_fc2_row", [1, D], BF16, kind="ExternalInput").ap()
    y_out = nc.dram_tensor("y", [NQ, D], F32, kind="ExternalOutput").ap()

    with tile.TileContext(nc) as tc:
        with (
            tc.tile_pool(name="persist", bufs=1) as persist,
            tc.tile_pool(name="small", bufs=8) as small,
            tc.tile_pool(name="lntmp", bufs=3) as lntmp,
        ):
            # --- constants ---
            ident = persist.tile([P, P], BF16)
            make_identity(nc, ident)
            ones_bf = persist.tile([1, P], BF16)
            nc.vector.memset(ones_bf, 1.0)
            # ones on partition row DH (=64) so the broadcast matmul's lhsT
            # base partition matches rec's (bass requires them equal)
            ones_f32 = persist.tile([DH + 1, DH], F32)
            nc.vector.memset(ones_f32, 1.0)
            eps_t = persist.tile([P, 1], F32)
            nc.vector.memset(eps_t, EPS)
            if with_biases:
                qkvb_c = persist.tile([P, 3 * KD], F32)
                nc.sync.dma_start(out=qkvb_c, in_=qkvb_col)
                qkvb_r = persist.tile([1, 3 * D], BF16)
                nc.sync.dma_start(out=qkvb_r, in_=qkvb_row)
                bproj_r = persist.tile([1, D], BF16)
                nc.sync.dma_start(out=bproj_r, in_=bproj_row)
                fc1b_c = persist.tile([P, FT], F32)
                nc.sync.dma_start(out=fc1b_c, in_=fc1b_col)
                bfc2_r = persist.tile([1, D], BF16)
                nc.sync.dma_start(out=bfc2_r, in_=bfc2_row)

            # --- persistent activations ---
            x_own = persist.tile([P, NQT, D], F32)     # own rows of x (residual 1)
            x2 = persist.tile([P, NQT, D], F32)        # x + attn_out (residual 2)
            h2T = persist.tile([P, KD, NQ], BF16)      # LN2(x2)^T
            gT = persist.tile([P, FT, NQ], BF16)       # gelu(fc1)^T

            pools = {"small": small, "lntmp": lntmp, "eps": eps_t}

            with tc.tile_pool(name="qkvout", bufs=1) as qkvout:
                QT = qkvout.tile([P, KD, NQ], BF16)       # Q^T [feat, q]
                KT = qkvout.tile([P, KD, N], BF16)        # K^T [feat, k]
                Vx = qkvout.tile([P, NT, H, DH + 1], BF16)  # V + ones col
                oT = qkvout.tile([P, KD, NQ], BF16)       # attn out^T [feat, q]
                nc.vector.memset(Vx[:, :, :, DH:DH + 1], 1.0)

                # ---------------- Phase 1: LN1 + h^T + QKV ----------------
                with (
                    tc.tile_pool(name="p1", bufs=1) as p1,
                    tc.tile_pool(name="xkv", bufs=4) as xkvp,
                    tc.tile_pool(name="wqkv", bufs=12) as wqp,
                    tc.psum_pool(name="tp_ps", bufs=2) as tp_ps,
                    tc.psum_pool(name="q_ps", bufs=2) as q_psp,
                    tc.psum_pool(name="kv_ps", bufs=2) as kv_psp,
                ):
                    hT = p1.tile([P, KD, N], BF16)

                    # load x tiles; own rows persist, kv rows stream
                    x_tiles = []
                    for i in range(NT):
                        if i < NQT:
                            dst = x_own[:, i, :]
                        else:
                            xs = xkvp.tile([P, D], F32, tag="xkv", name=f"xkv_{i}")
                            dst = xs
                        nc.sync.dma_start(out=dst, in_=x_in[i * P:(i + 1) * P, :])
                        x_tiles.append(dst)

                    # LN1 + transpose into hT (evacuate on DVE; ACT is the
                    # second-busiest engine, keep it free for exp/gelu)
                    def ln1_out(i, h_t):
                        def tcb(j, ps):
                            nc.vector.tensor_copy(
                                out=hT[:, j, i * P:(i + 1) * P], in_=ps)
                        _transpose_to(nc, tp_ps, ident, h_t, tcb, KD, idx0=i)

                    _layer_norm_tiles(nc, pools, x_tiles, ln1_out)

                    # weight slabs: Q cols, K cols, V cols as [P, D] slabs
                    def load_w_slabs(col0, tag):
                        slabs = []
                        for kt in range(KD):
                            w = wqp.tile([P, D], BF16, tag="wqkv",
                                         name=f"{tag}_{kt}")
                            nc.sync.dma_start(
                                out=w, in_=wqkv[kt * P:(kt + 1) * P, col0:col0 + D])
                            slabs.append(w)
                        return slabs

                    wq_s = load_w_slabs(0, "wq")
                    # Q^T [feat, q]: lhsT = Wq slab cols, rhs = h^T own cols
                    for m in range(KD):
                        qp = q_psp.tile([P, NQ], F32, tag="qps", name=f"qps_{m}")
                        for kt in range(KD):
                            nc.tensor.matmul(
                                qp, lhsT=wq_s[kt][:, m * P:(m + 1) * P],
                                rhs=hT[:, kt, 0:NQ],
                                start=(kt == 0), stop=(kt == KD - 1))
                        if with_biases:
                            nc.scalar.activation(
                                out=QT[:, m, :], in_=qp, func=AF.Identity,
                                bias=qkvb_c[:, m:m + 1], scale=1.0)
                        else:
                            nc.scalar.copy(out=QT[:, m, :], in_=qp)

                    wk_s = load_w_slabs(D, "wk")
                    for m in range(KD):
                        kp = kv_psp.tile([P, N], F32, tag="kvps", name=f"kps_{m}")
                        for c in range(2):
                            for kt in range(KD):
                                nc.tensor.matmul(
                                    kp[:, c * 512:(c + 1) * 512],
                                    lhsT=wk_s[kt][:, m * P:(m + 1) * P],
                                    rhs=hT[:, kt, c * 512:(c + 1) * 512],
                                    start=(kt == 0), stop=(kt == KD - 1))
                        if with_biases:
                            nc.scalar.activation(
                                out=KT[:, m, :], in_=kp, func=AF.Identity,
                                bias=qkvb_c[:, KD + m:KD + m + 1], scale=1.0)
                        else:
                            nc.scalar.copy(out=KT[:, m, :], in_=kp)

                    wv_s = load_w_slabs(2 * D, "wv")
                    # V [tok, feat]: lhsT = h^T tok cols, rhs = Wv slab
                    for i in range(NT):
                        vp = kv_psp.tile([P, D], F32, tag="kvps", name=f"vps_{i}")
                        for c in range(2):
                            if with_biases:
                                nc.tensor.matmul(
                                    vp[:, c * 512:(c + 1) * 512],
                                    lhsT=ones_bf[:, 0:P],
                                    rhs=qkvb_r[:, 2 * D + c * 512:
                                               2 * D + (c + 1) * 512],
                                    start=True, stop=False)
                            for kt in range(KD):
                                nc.tensor.matmul(
                                    vp[:, c * 512:(c + 1) * 512],
                                    lhsT=hT[:, kt, i * P:(i + 1) * P],
                                    rhs=wv_s[kt][:, c * 512:(c + 1) * 512],
                                    start=(not with_biases and kt == 0),
                                    stop=(kt == KD - 1))
                        nc.vector.tensor_copy(
                            out=Vx[:, i, :, 0:DH],
                            in_=vp.rearrange("p (h d) -> p h d", h=H))

                # ---------------- Phase 2: attention ----------------
                with (
                    tc.tile_pool(name="expp", bufs=24) as expp,
                    tc.tile_pool(name="recp", bufs=4) as recp,
                    tc.tile_pool(name="otmp", bufs=2) as otmpp,
                    tc.tile_pool(name="recd", bufs=4, space="DRAM") as recdp,
                    tc.psum_pool(name="s_ps", bufs=6) as s_psp,
                    tc.psum_pool(name="o_ps", bufs=2) as o_psp,
                ):
                    # heads processed in even/odd pairs: their score matmuls
                    # use disjoint PE row groups (K rows 0-63 vs 64-127), so
                    # interleaving lets the 16x32x32-subarray PE overlap them
                    for t in range(H // 2):
                        kt2 = t
                        ets = {0: [], 1: []}
                        for kt in range(NT):
                            for par in range(2):
                                h = 2 * t + par
                                po = par * DH
                                sp = s_psp.tile([P, NQ], F32, tag="sps",
                                                name=f"sps_{h}_{kt}")
                                nc.tensor.matmul(
                                    sp,
                                    lhsT=KT[po:po + DH, kt2,
                                            kt * P:(kt + 1) * P],
                                    rhs=QT[po:po + DH, kt2, :],
                                    start=True, stop=True)
                                et = expp.tile([P, NQ], BF16, tag="expt",
                                               name=f"expt_{h}_{kt}")
                                nc.scalar.activation(out=et, in_=sp,
                                                     func=AF.Exp, scale=SCALE)
                                ets[par].append(et)
                        for par in range(2):
                            h = 2 * t + par
                            op = o_psp.tile([DH + 1, NQ], F32, tag="ops",
                                            name=f"ops_{h}")
                            for kt in range(NT):
                                nc.tensor.matmul(op, lhsT=Vx[:, kt, h, :],
                                                 rhs=ets[par][kt],
                                                 start=(kt == 0),
                                                 stop=(kt == NT - 1))
                            # reciprocal of the sums row; DVE is lane-locked,
                            # keep it on partition DH (row 64) end-to-end
                            rec = recp.tile([DH + 1, NQ], F32, tag="rec",
                                            name=f"rec_{h}")
                            nc.vector.reciprocal(rec[DH:DH + 1, :],
                                                 op[DH:DH + 1, :])
                            # broadcast 1/sum across 64 partitions via a DRAM
                            # bounce (DMA replays the row with partition step
                            # 0) - cheaper than a PE ones-matmul + ACT copy
                            bnc = recdp.tile([1, NQ], F32, tag="recd",
                                             name=f"recd_{h}")
                            nc.sync.dma_start(out=bnc, in_=rec[DH:DH + 1, :])
                            bc_sb = recp.tile([DH, NQ], F32, tag="bcsb",
                                              name=f"bcsb_{h}")
                            nc.sync.dma_start(
                                out=bc_sb,
                                in_=bass.AP(tensor=bnc.tensor, offset=bnc.offset,
                                            ap=[[0, DH], [1, NQ]]))
                            if par == 0:
                                nc.vector.tensor_mul(out=oT[0:DH, kt2, :],
                                                     in0=op[0:DH, :],
                                                     in1=bc_sb)
                            else:
                                # odd heads land on partitions 64..127 of oT;
                                # DVE can't shift lanes - bounce via small DMA
                                ot = otmpp.tile([DH, NQ], BF16, tag="otmp",
                                                name=f"otmp_{h}")
                                nc.vector.tensor_mul(out=ot, in0=op[0:DH, :],
                                                     in1=bc_sb)
                                nc.sync.dma_start(out=oT[DH:P, kt2, :],
                                                  in_=ot)

                # ---------------- Phase 3: proj + residual + LN2 ----------------
                with (
                    tc.tile_pool(name="wproj", bufs=8) as wpp,
                    tc.psum_pool(name="pr_ps", bufs=2) as pr_psp,
                    tc.psum_pool(name="tp2_ps", bufs=2) as tp2_ps,
                ):
                    wp_s = []
                    for kt in range(KD):
                        w = wpp.tile([P, D], BF16, tag="wproj", name=f"wp_{kt}")
                        nc.sync.dma_start(out=w, in_=wproj[kt * P:(kt + 1) * P, :])
                        wp_s.append(w)
                    for m in range(NQT):
                        pp = pr_psp.tile([P, D], F32, tag="prps", name=f"prps_{m}")
                        for c in range(2):
                            if with_biases:
                                nc.tensor.matmul(
                                    pp[:, c * 512:(c + 1) * 512],
                                    lhsT=ones_bf[:, 0:P],
                                    rhs=bproj_r[:, c * 512:(c + 1) * 512],
                                    start=True, stop=False)
                            for kt in range(KD):
                                nc.tensor.matmul(
                                    pp[:, c * 512:(c + 1) * 512],
                                    lhsT=oT[:, kt, m * P:(m + 1) * P],
                                    rhs=wp_s[kt][:, c * 512:(c + 1) * 512],
                                    start=(not with_biases and kt == 0),
                                    stop=(kt == KD - 1))
                        nc.vector.scalar_tensor_tensor(
                            out=x2[:, m, :], in0=pp, scalar=1.0,
                            in1=x_own[:, m, :], op0=OP.mult, op1=OP.add)

                    # LN2 + transpose into h2T
                    def ln2_out(m, h_t):
                        def tcb(j, ps):
                            nc.vector.tensor_copy(
                                out=h2T[:, j, m * P:(m + 1) * P], in_=ps)
                        _transpose_to(nc, tp2_ps, ident, h_t, tcb, KD,
                                      idx0=100 + m)

                    _layer_norm_tiles(
                        nc, pools, [x2[:, m, :] for m in range(NQT)], ln2_out)

            # ---------------- Phase 4: MLP ----------------
            with (
                tc.tile_pool(name="wfc1", bufs=16) as wf1p,
                tc.tile_pool(name="wfc2", bufs=8) as wf2p,
                tc.tile_pool(name="yp", bufs=2) as yp,
                tc.psum_pool(name="a_ps", bufs=3) as a_psp,
                tc.psum_pool(name="o3_ps", bufs=2) as o3_psp,
            ):
                # fc1 + gelu -> gT, streamed in 4 column groups
                for fg in range(4):
                    f1_s = []
                    for kt in range(KD):
                        w = wf1p.tile([P, D], BF16, tag="wfc1",
                                      name=f"wf1_{fg}_{kt}")
                        nc.sync.dma_start(
                            out=w,
                            in_=wfc1[kt * P:(kt + 1) * P,
                                     fg * D:(fg + 1) * D])
                        f1_s.append(w)
                    for fl in range(KD):
                        f = fg * KD + fl
                        ap_ = a_psp.tile([P, NQ], F32, tag="aps",
                                         name=f"aps_{f}")
                        for kt in range(KD):
                            nc.tensor.matmul(
                                ap_, lhsT=f1_s[kt][:, fl * P:(fl + 1) * P],
                                rhs=h2T[:, kt, :],
                                start=(kt == 0), stop=(kt == KD - 1))
                        nc.scalar.activation(
                            out=gT[:, f, :], in_=ap_, func=AF.Gelu,
                            bias=(fc1b_c[:, f:f + 1] if with_biases else 0.0),
                            scale=1.0)

                # fc2 + residual, two passes of two token tiles each
                for pss in range(2):
                    ms = [2 * pss, 2 * pss + 1]
                    ops_ = {}
                    for m in ms:
                        o3 = o3_psp.tile([P, D], F32, tag="o3ps",
                                         name=f"o3ps_{m}")
                        if with_biases:
                            for c in range(2):
                                nc.tensor.matmul(
                                    o3[:, c * 512:(c + 1) * 512],
                                    lhsT=ones_bf[:, 0:P],
                                    rhs=bfc2_r[:, c * 512:(c + 1) * 512],
                                    start=True, stop=False)
                        ops_[m] = o3
                    for kt in range(FT):
                        w2 = wf2p.tile([P, D], BF16, tag="wfc2",
                                       name=f"wf2_{pss}_{kt}")
                        nc.sync.dma_start(out=w2,
                                          in_=wfc2[kt * P:(kt + 1) * P, :])
                        for m in ms:
                            for c in range(2):
                                nc.tensor.matmul(
                                    ops_[m][:, c * 512:(c + 1) * 512],
                                    lhsT=gT[:, kt, m * P:(m + 1) * P],
                                    rhs=w2[:, c * 512:(c + 1) * 512],
                                    start=(not with_biases and kt == 0),
                                    stop=(kt == FT - 1))
                    for m in ms:
                        y_t = yp.tile([P, D], F32, tag="y", name=f"y_{m}")
                        nc.vector.scalar_tensor_tensor(
                            out=y_t, in0=ops_[m], scalar=1.0,
                            in1=x2[:, m, :], op0=OP.mult, op1=OP.add)
                        nc.sync.dma_start(out=y_out[m * P:(m + 1) * P, :],
                                          in_=y_t)
    nc.compile()  # bacc passes: split multi-waits (HW allows 1/inst), regalloc
    return nc


_NC_CACHE = {}


def _get_nc(with_biases=True):
    if with_biases not in _NC_CACHE:
        _NC_CACHE[with_biases] = build_program(with_biases)
    return _NC_CACHE[with_biases]


def make_in_maps(x, ln1_g, ln1_b, ln2_g, ln2_b, w_qkv, w_proj, b_proj,
                 w_fc1, b_fc1, w_fc2, b_fc2):
    x = np.asarray(x, dtype=np.float32)
    ln1_g = np.asarray(ln1_g, np.float32); ln1_b = np.asarray(ln1_b, np.float32)
    ln2_g = np.asarray(ln2_g, np.float32); ln2_b = np.asarray(ln2_b, np.float32)
    w_qkv = np.asarray(w_qkv, np.float32); w_proj = np.asarray(w_proj, np.float32)
    b_proj = np.asarray(b_proj, np.float32)
    w_fc1 = np.asarray(w_fc1, np.float32); b_fc1 = np.asarray(b_fc1, np.float32)
    w_fc2 = np.asarray(w_fc2, np.float32); b_fc2 = np.asarray(b_fc2, np.float32)

    bf = ml_dtypes.bfloat16

    # fold LN affine params into the following matmul (exact algebra)
    w_qkv_eff = (ln1_g[:, None] * w_qkv)
    qkv_bias = ln1_b @ w_qkv                      # [3D]
    w_fc1_eff = (ln2_g[:, None] * w_fc1)
    fc1_bias = b_fc1 + ln2_b @ w_fc1              # [F]

    common = {
        "w_qkv": w_qkv_eff.astype(bf),
        "qkv_b_col": np.ascontiguousarray(
            qkv_bias.reshape(3 * KD, P).T.astype(np.float32)),
        "qkv_b_row": qkv_bias.reshape(1, 3 * D).astype(bf),
        "w_proj": w_proj.astype(bf),
        "b_proj_row": b_proj.reshape(1, D).astype(bf),
        "w_fc1": w_fc1_eff.astype(bf),
        "fc1_b_col": np.ascontiguousarray(
            fc1_bias.reshape(FT, P).T.astype(np.float32)),
        "w_fc2": w_fc2.astype(bf),
        "b_fc2_row": b_fc2.reshape(1, D).astype(bf),
    }

    in_maps = []
    for c in range(8):
        b = c // 2
        q0 = (c % 2) * NQ
        xb = x[b]
        x_roll = np.ascontiguousarray(
            np.concatenate([xb[q0:q0 + NQ], xb[NQ - q0:2 * NQ - q0]], axis=0))
        in_maps.append({"x_in": x_roll, **common})

    with_biases = not (
        np.all(qkv_bias == 0) and np.all(b_proj == 0)
        and np.all(fc1_bias == 0) and np.all(b_fc2 == 0))
    return in_maps, with_biases


def kernel(**inputs):
    global LAST_RESULTS
    in_maps, with_biases = make_in_maps(**inputs)
    nc = _get_nc(with_biases)
    res = run_bass_kernel_spmd(nc, in_maps, core_ids=list(range(8)),
                               trace=TRACE)
    LAST_RESULTS = res

    out = np.empty((4, N, D), np.float32)
    for c in range(8):
        b = c // 2
        q0 = (c % 2) * NQ
        out[b, q0:q0 + NQ] = res.results[c]["y"]
    return out
